# revision 5
# baseline (speedup 1.0000x reference)
"""Distributed multi-head attention for TRN2 (8 NeuronCores).

Reference computation (per batch b):
    qkv = x @ w_qkv.T                         # (N, 3C)
    q, k, v = split/reshape to (H, N, D)
    attn = softmax(q @ k.T * D**-0.5)         # per head
    out = (attn @ v) reassembled to (N, C)
    out = out @ w_proj.T + b_proj

Sharding: 8 cores = 4 batches x 2 query-halves. Each core computes k/v
for all 2048 tokens of its batch (duplicated across the 2 cores of a
batch - cheaper than communicating), q for its own 1024 tokens, the
full attention for all 12 heads over its 1024 queries, and the output
projection. No collectives.

Layout strategy (all chosen so no on-chip transposes are needed):
  - host passes x^T and w_qkv^T so projections contract over partitions
  - q,k are produced "d-major" ([head-dim, tokens]) via out^T-form
    matmuls; scores are computed transposed ([keys, queries]) which is
    exactly the layout attn@v consumes as its stationary-side operand
  - softmax needs no max-subtraction (scores ~ N(0,1), fp32 exp range)
  - the denominator rides along as a ones-column appended to v (M=65
    matmuls); normalization uses a K=1 ones-matmul to broadcast 1/denom
    across partitions
  - matmul dtypes: f32r (tf32-like, full PE rate at N>=256) for qkv
    projection + scores; bf16 for probs @ v and the output projection

Self-contained: hardcodes B=4, N=2048, C=768, H=12, D=64.
"""

import numpy as np
import ml_dtypes

import concourse.bass as bass
import concourse.mybir as mybir
from concourse import bacc
from concourse.tile import TileContext
from concourse.bass_utils import run_bass_kernel_spmd

F32 = mybir.dt.float32
F32R = mybir.dt.float32r
BF16 = mybir.dt.bfloat16
EXP = mybir.ActivationFunctionType.Exp

B, N, C = 4, 2048, 768
H, D = 12, 64
SCALE = float(D) ** -0.5  # 0.125
NQ = N // 2  # queries per core: 1024
CB = C // 128  # 6 c-chunks
TB = N // 128  # 16 token blocks
HB = H // 2  # 6 head pairs
VW = H * (D + 1)  # 780: v block width with ones columns

N_CORES = 8


def _build():
    nc = bacc.Bacc(None, target_bir_lowering=False)

    xT = nc.declare_dram_parameter("xT", [C, N], F32R, isOutput=False)
    wqkvT = nc.declare_dram_parameter("wqkvT", [C, 3 * C], F32R, isOutput=False)
    wprojT = nc.declare_dram_parameter("wprojT", [C, C], BF16, isOutput=False)
    bias = nc.declare_dram_parameter("bias", [C, 1], F32, isOutput=False)
    outT = nc.declare_dram_parameter("outT", [C, NQ], F32, isOutput=True)

    with TileContext(nc) as tc:
        with (
            tc.tile_pool(name="per", bufs=1) as per,
            tc.tile_pool(name="psO", bufs=2, space="PSUM") as psO,
        ):
            # ---- tiles that live through all phases -------------------
            qT_sb = per.tile([128, CB * NQ], F32R)  # q^T  [2 heads/blk, 1024]
            kT_sb = per.tile([128, CB * N], F32R)  # k^T  [2 heads/blk, 2048]
            vaug_sb = per.tile([128, TB * VW], BF16)  # v + ones cols, bf16
            bias_sb = per.tile([128, CB], F32)
            ones_sb = per.tile([1, 64], BF16)

            nc.vector.memset(ones_sb[:, :], 1.0)
            # ones columns of vaug: col 64 of each 65-wide head slot
            vaug_ones = vaug_sb[:, :].rearrange(
                "p (t h x) -> p t h x", t=TB, h=H, x=D + 1
            )[:, :, :, D : D + 1]
            nc.vector.memset(vaug_ones, 1.0)

            for ci in range(CB):
                nc.sync.dma_start(
                    out=bias_sb[:, ci : ci + 1],
                    in_=bias[ci * 128 : (ci + 1) * 128, :],
                )

            # ---- phase 1: qkv projection ------------------------------
            with (
                tc.tile_pool(name="wq", bufs=1) as wq_pool,
                tc.tile_pool(name="xt", bufs=2) as xt_pool,
                tc.tile_pool(name="psA", bufs=3, space="PSUM") as psA,
            ):
                wqkv_sb = wq_pool.tile([128, CB * 3 * C], F32R)
                for ci in range(CB):
                    nc.sync.dma_start(
                        out=wqkv_sb[:, ci * 3 * C : (ci + 1) * 3 * C],
                        in_=wqkvT[ci * 128 : (ci + 1) * 128, :],
                    )

                def wq(ci, o0, width):
                    """lhsT slice of wqkvT chunk ci, out-dims [o0, o0+width)"""
                    base = ci * 3 * C + o0
                    return wqkv_sb[:, base : base + width]

                # xT streamed in 512-token chunks; within a chunk, all
                # q/k o-blocks (out^T form) and v t-blocks (natural form)
                for tch in range(4):
                    t0 = tch * 512
                    xt = xt_pool.tile([128, CB * 512], F32R, tag="xt")
                    for ci in range(CB):
                        nc.sync.dma_start(
                            out=xt[:, ci * 512 : (ci + 1) * 512],
                            in_=xT[ci * 128 : (ci + 1) * 128, t0 : t0 + 512],
                        )

                    def xs(ci, off=0, width=512):
                        base = ci * 512 + off
                        return xt[:, base : base + width]

                    # q^T blocks (queries are xT cols [0, 1024))
                    if tch < 2:
                        for ob in range(CB):
                            ps = psA.tile([128, 512], F32, tag="psA")
                            for ci in range(CB):
                                nc.tensor.matmul(
                                    ps[:, :],
                                    wq(ci, ob * 128, 128),
                                    xs(ci),
                                    start=(ci == 0),
                                    stop=(ci == CB - 1),
                                )
                            nc.vector.tensor_copy(
                                qT_sb[:, ob * NQ + t0 : ob * NQ + t0 + 512], ps[:, :]
                            )
                    # k^T blocks (all 2048 tokens)
                    for ob in range(CB):
                        ps = psA.tile([128, 512], F32, tag="psA")
                        for ci in range(CB):
                            nc.tensor.matmul(
                                ps[:, :],
                                wq(ci, C + ob * 128, 128),
                                xs(ci),
                                start=(ci == 0),
                                stop=(ci == CB - 1),
                            )
                        nc.vector.tensor_copy(
                            kT_sb[:, ob * N + t0 : ob * N + t0 + 512], ps[:, :]
                        )
                    # v blocks, natural layout: out[tok, vdims]
                    for tb in range(4):
                        t128 = tch * 4 + tb
                        for o0, w in [(0, 512), (512, 256)]:
                            ps = psA.tile([128, 512], F32, tag="psA")
                            for ci in range(CB):
                                nc.tensor.matmul(
                                    ps[:, :w],
                                    xs(ci, tb * 128, 128),
                                    wq(ci, 2 * C + o0, w),
                                    start=(ci == 0),
                                    stop=(ci == CB - 1),
                                )
                            nh = w // D  # heads in this chunk
                            src = ps[:, :w].rearrange("p (h x) -> p h x", x=D)
                            h0 = o0 // D
                            base = t128 * VW + h0 * (D + 1)
                            dst = vaug_sb[:, base : base + nh * (D + 1)].rearrange(
                                "p (h x) -> p h x", x=D + 1
                            )[:, :, :D]
                            nc.vector.tensor_copy(dst, src)

            # ---- phases 2+3 pools -------------------------------------
            with (
                tc.tile_pool(name="p23", bufs=1) as p23,
                tc.tile_pool(name="hp", bufs=4) as hp,
                tc.tile_pool(name="mi", bufs=2) as mi,
                tc.tile_pool(name="op", bufs=2) as op_pool,
                tc.tile_pool(name="psS", bufs=2, space="PSUM") as psS,
            ):
                attnT_sb = p23.tile([128, CB * NQ], BF16)  # attn out^T
                wproj_sb = p23.tile([128, CB * C], BF16)
                for ci in range(CB):
                    nc.sync.dma_start(
                        out=wproj_sb[:, ci * C : (ci + 1) * C],
                        in_=wprojT[ci * 128 : (ci + 1) * 128, :],
                    )

                # ---- phase 2: attention, one head pair at a time ------
                for hb in range(HB):
                    accs = [
                        psO.tile([128, NQ], F32, tag="psO", name=f"acc{hb}_{i}")
                        for i in range(2)
                    ]
                    for kb in range(TB):
                        sc = [
                            psS.tile(
                                [128, NQ], F32, tag="psS", name=f"sc{hb}_{kb}_{i}"
                            )
                            for i in range(2)
                        ]
                        for hh in range(2):
                            p0 = 64 * hh
                            for qc in range(2):
                                nc.tensor.matmul(
                                    sc[hh][:, qc * 512 : (qc + 1) * 512],
                                    kT_sb[
                                        p0 : p0 + 64,
                                        hb * N + kb * 128 : hb * N + (kb + 1) * 128,
                                    ],
                                    qT_sb[
                                        p0 : p0 + 64,
                                        hb * NQ + qc * 512 : hb * NQ + (qc + 1) * 512,
                                    ],
                                    start=True,
                                    stop=True,
                                )
                        for hh in range(2):
                            h = 2 * hb + hh
                            pb = hp.tile([128, NQ], BF16, tag="probs")
                            nc.scalar.activation(
                                pb[:, :], sc[hh][:, :], EXP, scale=SCALE
                            )
                            vs = kb * VW + h * (D + 1)
                            for qc in range(2):
                                nc.tensor.matmul(
                                    accs[hh][0:65, qc * 512 : (qc + 1) * 512],
                                    vaug_sb[:, vs : vs + D + 1],
                                    pb[:, qc * 512 : (qc + 1) * 512],
                                    start=(kb == 0),
                                    stop=(kb == TB - 1),
                                )
                    # epilogue: normalize by the ones-column sums
                    for hh in range(2):
                        acc = accs[hh]
                        rec = mi.tile([1, NQ], F32, tag="rec")
                        nc.vector.reciprocal(rec[:, :], acc[64:65, :])
                        row = mi.tile([1, NQ], BF16, tag="row")
                        nc.vector.tensor_copy(row[:, :], rec[:, :])
                        psb = psS.tile(
                            [128, NQ], F32, tag="psS", name=f"psb{hb}_{hh}"
                        )
                        for qc in range(2):
                            nc.tensor.matmul(
                                psb[0:64, qc * 512 : (qc + 1) * 512],
                                ones_sb[:, :],
                                row[:, qc * 512 : (qc + 1) * 512],
                                start=True,
                                stop=True,
                            )
                        bc = mi.tile([64, NQ], F32, tag="bc")
                        nc.vector.tensor_copy(bc[:, :], psb[0:64, :])
                        nc.vector.tensor_mul(
                            attnT_sb[
                                64 * hh : 64 * hh + 64, hb * NQ : (hb + 1) * NQ
                            ],
                            acc[0:64, :],
                            bc[:, :],
                        )

                # ---- phase 3: output projection (out^T form) ----------
                for ob in range(CB):
                    ps = psO.tile([128, NQ], F32, tag="psO", name=f"prj{ob}")
                    for cb in range(CB):
                        for qc in range(2):
                            nc.tensor.matmul(
                                ps[:, qc * 512 : (qc + 1) * 512],
                                wproj_sb[
                                    :, cb * C + ob * 128 : cb * C + (ob + 1) * 128
                                ],
                                attnT_sb[
                                    :, cb * NQ + qc * 512 : cb * NQ + (qc + 1) * 512
                                ],
                                start=(cb == 0),
                                stop=(cb == CB - 1),
                            )
                    ot = op_pool.tile([128, NQ], F32, tag="out")
                    nc.vector.tensor_scalar_add(
                        ot[:, :], ps[:, :], bias_sb[:, ob : ob + 1]
                    )
                    nc.sync.dma_start(
                        out=outT[ob * 128 : (ob + 1) * 128, :], in_=ot[:, :]
                    )

    nc.finalize()
    return nc


_NC_CACHE = []


def _get_nc():
    if not _NC_CACHE:
        _NC_CACHE.append(_build())
    return _NC_CACHE[0]


def kernel(x, w_qkv, w_proj, b_proj):
    x = np.asarray(x, dtype=np.float32)
    w_qkv = np.asarray(w_qkv, dtype=np.float32)
    w_proj = np.asarray(w_proj, dtype=np.float32)
    b_proj = np.asarray(b_proj, dtype=np.float32)

    nc = _get_nc()

    wqkvT = np.ascontiguousarray(w_qkv.T)
    wprojT = np.ascontiguousarray(w_proj.T).astype(ml_dtypes.bfloat16)
    bias = np.ascontiguousarray(b_proj.reshape(C, 1))

    in_maps = []
    for core in range(N_CORES):
        b, half = divmod(core, 2)
        # own 1024 query tokens first, then the other half (key order
        # within attention is permutation-invariant)
        mine = x[b, half * NQ : (half + 1) * NQ].T
        other = x[b, (1 - half) * NQ : (2 - half) * NQ].T
        xTc = np.ascontiguousarray(np.concatenate([mine, other], axis=1))
        in_maps.append({"xT": xTc, "wqkvT": wqkvT, "wprojT": wprojT, "bias": bias})

    res = run_bass_kernel_spmd(nc, in_maps, core_ids=list(range(N_CORES)))

    out = np.empty((B, N, C), dtype=np.float32)
    for core in range(N_CORES):
        b, half = divmod(core, 2)
        out[b, half * NQ : (half + 1) * NQ, :] = res.results[core]["outT"].T
    return out


# revision 14
# speedup vs baseline: 1.7469x; 1.7469x over previous
"""Distributed multi-head attention for TRN2 (8 NeuronCores).

Reference computation (per batch b):
    qkv = x @ w_qkv.T                         # (N, 3C)
    q, k, v = split/reshape to (H, N, D)
    attn = softmax(q @ k.T * D**-0.5)         # per head
    out = (attn @ v) reassembled to (N, C)
    out = out @ w_proj.T + b_proj

Sharding: 8 cores = 4 batches x 2 query-halves. Each core computes k/v
for all 2048 tokens of its batch (duplicated across the 2 cores of a
batch - cheaper than communicating), q for its own 1024 tokens, the
full attention for all 12 heads over its 1024 queries, and the output
projection. No collectives.

Layout strategy (all chosen so no on-chip transposes are needed):
  - host passes x^T and w_qkv^T so projections contract over partitions
  - q,k are produced "d-major" ([head-dim, tokens]) via out^T-form
    matmuls; scores are computed transposed ([keys, queries]) which is
    exactly the layout attn@v consumes as its stationary-side operand
  - softmax needs no max-subtraction (scores ~ N(0,1), fp32 exp range)
  - the denominator rides along as a ones-column appended to v (M=65
    matmuls); normalization uses a K=1 ones-matmul to broadcast 1/denom
    across partitions
  - all matmuls in bf16 (PSUM accumulation is fp32); softmax exp runs
    on the scalar (ACT) engine from PSUM f32, writing bf16 probs

Schedule: the ACT engine (softmax exp) is the steady-state bottleneck;
everything else is arranged to hide under it:
  - per head pair, scores(kb)/exp(kb)/attn@v(kb-1) are software-
    pipelined; the two heads' score matmuls alternate PE row groups
    (base partitions 0/64) so they run concurrently (row tiling)
  - the v projection overlaps head-pair 0 (it only needs v block kb
    at step kb), in a dedicated PSUM scope
  - each pair's normalization epilogue is split: DVE part at pair end
    (frees the PSUM accumulators), PE part deferred into the next pair

Self-contained: hardcodes B=4, N=2048, C=768, H=12, D=64.
"""

import numpy as np
import ml_dtypes

import concourse.bass as bass
import concourse.mybir as mybir
from concourse import bacc
from concourse.tile import TileContext
from concourse.bass_utils import run_bass_kernel_spmd

F32 = mybir.dt.float32
BF16 = mybir.dt.bfloat16
EXP = mybir.ActivationFunctionType.Exp

B, N, C = 4, 2048, 768
H, D = 12, 64
SCALE = float(D) ** -0.5  # 0.125
NQ = N // 2  # queries per core: 1024
CB = C // 128  # 6 c-chunks
TB = N // 128  # 16 token blocks
HB = H // 2  # 6 head pairs
VW = H * (D + 1)  # 780: v block width with ones columns

N_CORES = 8


def _build():
    nc = bacc.Bacc(None, target_bir_lowering=False)

    xT = nc.declare_dram_parameter("xT", [C, N], BF16, isOutput=False)
    wqkvT = nc.declare_dram_parameter("wqkvT", [C, 3 * C], BF16, isOutput=False)
    wprojT = nc.declare_dram_parameter("wprojT", [C, C], BF16, isOutput=False)
    bias = nc.declare_dram_parameter("bias", [C, 1], F32, isOutput=False)
    outT = nc.declare_dram_parameter("outT", [C, NQ], F32, isOutput=True)

    with TileContext(nc) as tc:
        with (
            tc.tile_pool(name="per", bufs=1) as per,
            tc.tile_pool(name="p23", bufs=1) as p23,
            tc.tile_pool(name="hp", bufs=6) as hp,
            tc.tile_pool(name="mi", bufs=2) as mi,
            tc.tile_pool(name="op", bufs=2) as op_pool,
            tc.tile_pool(name="psO", bufs=2, space="PSUM") as psO,
        ):
            # ---- persistent tiles -------------------------------------
            qT_sb = per.tile([128, CB * NQ], BF16)  # q^T  [2 heads/blk, 1024]
            kT_sb = per.tile([128, CB * N], BF16)  # k^T  [2 heads/blk, 2048]
            vaug_sb = per.tile([128, TB * VW], BF16)  # v + ones cols
            bias_sb = per.tile([128, CB], F32)
            ones_sb = per.tile([1, 64], BF16)
            attnT_sb = p23.tile([128, CB * NQ], BF16)  # attn out^T
            wproj_sb = p23.tile([128, CB * C], BF16)

            nc.vector.memset(ones_sb[:, :], 1.0)
            # ones columns of vaug: col 64 of each 65-wide head slot
            vaug_ones = vaug_sb[:, :].rearrange(
                "p (t h x) -> p t h x", t=TB, h=H, x=D + 1
            )[:, :, :, D : D + 1]
            nc.vector.memset(vaug_ones, 1.0)

            for ci in range(CB):
                nc.sync.dma_start(
                    out=bias_sb[:, ci : ci + 1],
                    in_=bias[ci * 128 : (ci + 1) * 128, :],
                )
                nc.sync.dma_start(
                    out=wproj_sb[:, ci * C : (ci + 1) * C],
                    in_=wprojT[ci * 128 : (ci + 1) * 128, :],
                )

            # weights + activations pools, closed once the v projection
            # (inside head-pair 0) has consumed them
            wqxt = (tc.tile_pool(name="wq", bufs=1), tc.tile_pool(name="xt", bufs=4))
            wq_pool = wqxt[0].__enter__()
            xt_pool = wqxt[1].__enter__()

            wqkv_sb = wq_pool.tile([128, CB * 3 * C], BF16)
            for ci in range(CB):
                nc.sync.dma_start(
                    out=wqkv_sb[:, ci * 3 * C : (ci + 1) * 3 * C],
                    in_=wqkvT[ci * 128 : (ci + 1) * 128, :],
                )

            xts = []
            for tch in range(4):
                xt = xt_pool.tile([128, CB * 512], BF16, tag="xt", name=f"xt{tch}")
                for ci in range(CB):
                    nc.sync.dma_start(
                        out=xt[:, ci * 512 : (ci + 1) * 512],
                        in_=xT[ci * 128 : (ci + 1) * 128, tch * 512 : (tch + 1) * 512],
                    )
                xts.append(xt)

            def wq(ci, o0, width):
                base = ci * 3 * C + o0
                return wqkv_sb[:, base : base + width]

            # ---- phase 1: k^T / q^T projections -----------------------
            # token-chunk-major so compute on chunk 0 starts while later
            # chunks are still in DMA; low head-pair blocks first.
            with tc.tile_pool(name="psA", bufs=3, space="PSUM") as psA:
                for tch in range(4):
                    t0 = tch * 512
                    for ob in range(CB):
                        ps = psA.tile([128, 512], F32, tag="psA", name=f"k{ob}_{tch}")
                        for ci in range(CB):
                            nc.tensor.matmul(
                                ps[:, :],
                                wq(ci, C + ob * 128, 128),
                                xts[tch][:, ci * 512 : (ci + 1) * 512],
                                start=(ci == 0),
                                stop=(ci == CB - 1),
                            )
                        nc.vector.tensor_copy(
                            kT_sb[:, ob * N + t0 : ob * N + t0 + 512], ps[:, :]
                        )
                        if tch < 2:
                            ps = psA.tile(
                                [128, 512], F32, tag="psA", name=f"q{ob}_{tch}"
                            )
                            for ci in range(CB):
                                nc.tensor.matmul(
                                    ps[:, :],
                                    wq(ci, ob * 128, 128),
                                    xts[tch][:, ci * 512 : (ci + 1) * 512],
                                    start=(ci == 0),
                                    stop=(ci == CB - 1),
                                )
                            nc.vector.tensor_copy(
                                qT_sb[:, ob * NQ + t0 : ob * NQ + t0 + 512], ps[:, :]
                            )

            def v_unit(vpool, t128, o0, w):
                """one v-projection unit: 128 tokens x [o0, o0+w) v-dims,
                written (bf16) into the vaug slot layout"""
                tch, tb = divmod(t128, 4)
                ps = vpool.tile(
                    [128, 512], F32, tag="psV", bufs=2, name=f"v{t128}_{o0}"
                )
                for ci in range(CB):
                    nc.tensor.matmul(
                        ps[:, :w],
                        xts[tch][:, ci * 512 + tb * 128 : ci * 512 + (tb + 1) * 128],
                        wq(ci, 2 * C + o0, w),
                        start=(ci == 0),
                        stop=(ci == CB - 1),
                    )
                nh = w // D
                src = ps[:, :w].rearrange("p (h x) -> p h x", x=D)
                h0 = o0 // D
                base = t128 * VW + h0 * (D + 1)
                dst = vaug_sb[:, base : base + nh * (D + 1)].rearrange(
                    "p (h x) -> p h x", x=D + 1
                )[:, :, :D]
                nc.vector.tensor_copy(dst, src)

            # ---- phase 2 helpers --------------------------------------
            def epi_pe(pool, bufs, hb_, outs_):
                """PE part of pair hb_'s normalization epilogue"""
                for hh_ in range(2):
                    cpy_, row_ = outs_[hh_]
                    psb = pool.tile(
                        [128, NQ], F32, tag=pool.name + "T", bufs=bufs,
                        name=f"psb{hb_}_{hh_}",
                    )
                    for qc_ in range(2):
                        nc.tensor.matmul(
                            psb[0:64, qc_ * 512 : (qc_ + 1) * 512],
                            ones_sb[:, :],
                            row_[:, qc_ * 512 : (qc_ + 1) * 512],
                            start=True,
                            stop=True,
                        )
                    nc.vector.tensor_mul(
                        attnT_sb[
                            64 * hh_ : 64 * hh_ + 64, hb_ * NQ : (hb_ + 1) * NQ
                        ],
                        psb[0:64, :],
                        cpy_[:, :],
                    )

            def emit_pair(pool, bufs, hb, pend, filler=None):
                """One head pair: scores/exp/attn@v software pipeline.
                filler(kb, slot) emits extra PE work inside the steady
                state. Returns this pair's deferred epilogue state."""
                accs = [
                    psO.tile([128, NQ], F32, tag="psO", name=f"acc{hb}_{i}")
                    for i in range(2)
                ]
                prev = None
                for kb in range(TB):
                    sxy = []
                    for qc in range(2):
                        sc = pool.tile(
                            [128, NQ], F32, tag=pool.name + "T", bufs=bufs,
                            name=f"sc{hb}_{kb}_{qc}",
                        )
                        for hh in range(2):
                            p0 = 64 * hh
                            nc.tensor.matmul(
                                sc[:, hh * 512 : (hh + 1) * 512],
                                kT_sb[
                                    p0 : p0 + 64,
                                    hb * N + kb * 128 : hb * N + (kb + 1) * 128,
                                ],
                                qT_sb[
                                    p0 : p0 + 64,
                                    hb * NQ + qc * 512 : hb * NQ + (qc + 1) * 512,
                                ],
                                start=True,
                                stop=True,
                            )
                        sxy.append(sc)
                        if filler is not None:
                            filler(kb, qc)
                    if prev is not None:
                        pkb, ppb = prev
                        for hh in range(2):
                            vs = pkb * VW + (2 * hb + hh) * (D + 1)
                            for qc in range(2):
                                nc.tensor.matmul(
                                    accs[hh][0:65, qc * 512 : (qc + 1) * 512],
                                    vaug_sb[:, vs : vs + D + 1],
                                    ppb[qc][:, hh * 512 : (hh + 1) * 512],
                                    start=(pkb == 0),
                                    stop=(pkb == TB - 1),
                                )
                    pbs = []
                    for qc in range(2):
                        pb = hp.tile([128, NQ], BF16, tag="probs")
                        nc.scalar.activation(pb[:, :], sxy[qc][:, :], EXP, scale=SCALE)
                        pbs.append(pb)
                    prev = (kb, pbs)
                    if kb == 2 and pend is not None:
                        epi_pe(pool, bufs, *pend)
                        pend = None
                # drain: attn@v for the last k-block
                pkb, ppb = prev
                for hh in range(2):
                    vs = pkb * VW + (2 * hb + hh) * (D + 1)
                    for qc in range(2):
                        nc.tensor.matmul(
                            accs[hh][0:65, qc * 512 : (qc + 1) * 512],
                            vaug_sb[:, vs : vs + D + 1],
                            ppb[qc][:, hh * 512 : (hh + 1) * 512],
                            start=False,
                            stop=True,
                        )
                # epilogue DVE part: drain accumulators to SBUF + 1/denom
                outs = []
                for hh in range(2):
                    acc = accs[hh]
                    cpy = mi.tile([64, NQ], F32, tag="cpy")
                    nc.vector.tensor_copy(cpy[:, :], acc[0:64, :])
                    den = mi.tile([1, NQ], F32, tag="den")
                    nc.vector.tensor_copy(den[:, :], acc[64:65, :])
                    rec = mi.tile([1, NQ], F32, tag="rec")
                    nc.vector.reciprocal_approx_fast(rec[:, :], den[:, :])
                    row = mi.tile([1, NQ], BF16, tag="row")
                    nc.vector.tensor_copy(row[:, :], rec[:, :])
                    outs.append((cpy, row))
                return (hb, outs)

            # ---- phase 2: pair 0 with the v projection interleaved ----
            # single score slot (bufs=1) frees two PSUM banks for the v
            # units; v block kb is produced in step kb, just before
            # attn@v(kb) consumes it in step kb+1
            with tc.tile_pool(name="ps0", bufs=1, space="PSUM") as ps0:

                def fill_v(kb, qc):
                    v_unit(ps0, kb, 0 if qc == 0 else 512, 512 if qc == 0 else 256)

                pend = emit_pair(ps0, 1, 0, None, filler=fill_v)

            wqxt[1].__exit__(None, None, None)
            wqxt[0].__exit__(None, None, None)

            # ---- phase 2: pairs 1..5 + phase 3 ------------------------
            with tc.tile_pool(name="psS", bufs=2, space="PSUM") as psS:
                for hb in range(1, HB):
                    pend = emit_pair(psS, 2, hb, pend)
                epi_pe(psS, 2, *pend)

                # ---- phase 3: output projection (out^T form) ----------
                for ob in range(CB):
                    ps = psO.tile([128, NQ], F32, tag="psO", name=f"prj{ob}")
                    for cb in range(CB):
                        for qc in range(2):
                            nc.tensor.matmul(
                                ps[:, qc * 512 : (qc + 1) * 512],
                                wproj_sb[
                                    :, cb * C + ob * 128 : cb * C + (ob + 1) * 128
                                ],
                                attnT_sb[
                                    :, cb * NQ + qc * 512 : cb * NQ + (qc + 1) * 512
                                ],
                                start=(cb == 0),
                                stop=(cb == CB - 1),
                            )
                    ot = op_pool.tile([128, NQ], F32, tag="out")
                    nc.vector.tensor_scalar_add(
                        ot[:, :], ps[:, :], bias_sb[:, ob : ob + 1]
                    )
                    nc.sync.dma_start(
                        out=outT[ob * 128 : (ob + 1) * 128, :], in_=ot[:, :]
                    )

    nc.finalize()
    return nc


_NC_CACHE = []


def _get_nc():
    if not _NC_CACHE:
        _NC_CACHE.append(_build())
    return _NC_CACHE[0]


def kernel(x, w_qkv, w_proj, b_proj):
    x = np.asarray(x, dtype=np.float32)
    w_qkv = np.asarray(w_qkv, dtype=np.float32)
    w_proj = np.asarray(w_proj, dtype=np.float32)
    b_proj = np.asarray(b_proj, dtype=np.float32)

    nc = _get_nc()

    wqkvT = np.ascontiguousarray(w_qkv.T).astype(ml_dtypes.bfloat16)
    wprojT = np.ascontiguousarray(w_proj.T).astype(ml_dtypes.bfloat16)
    bias = np.ascontiguousarray(b_proj.reshape(C, 1))

    in_maps = []
    for core in range(N_CORES):
        b, half = divmod(core, 2)
        # own 1024 query tokens first, then the other half (key order
        # within attention is permutation-invariant)
        mine = x[b, half * NQ : (half + 1) * NQ].T
        other = x[b, (1 - half) * NQ : (2 - half) * NQ].T
        xTc = np.ascontiguousarray(np.concatenate([mine, other], axis=1)).astype(
            ml_dtypes.bfloat16
        )
        in_maps.append({"xT": xTc, "wqkvT": wqkvT, "wprojT": wprojT, "bias": bias})

    res = run_bass_kernel_spmd(nc, in_maps, core_ids=list(range(N_CORES)))

    out = np.empty((B, N, C), dtype=np.float32)
    for core in range(N_CORES):
        b, half = divmod(core, 2)
        out[b, half * NQ : (half + 1) * NQ, :] = res.results[core]["outT"].T
    return out
